# revision 7
# baseline (speedup 1.0000x reference)
"""2-layer GCN (gather/scatter message passing) on 8 trn2 NeuronCores.

Strategy (per sharding hint): nodes (and their incoming edges) are
partitioned across the 8 cores by dst-node range; each core computes
x@W1 for its node slice, slices are exchanged via AllGather (the
16-dim halo exchange), and each core aggregates messages for its dst
range twice (layer 1 and layer 2). Weight matrices are replicated.

On-chip per pass: the gather h[src] runs on GPSIMD (ap_gather) from
feature-major tables [16 feats x src-chunk] replicated per 16-partition
group; messages are weighted (DVE mul), prefix-summed along the
dst-sorted edge stream (DVE tensor_tensor_scan), and per-dst segment
sums are extracted by gathering the prefix at segment-end boundaries
and differencing (scan-diff).  Partials from the 8 groups are summed
with one PE matmul against a 0/1 selection matrix.
"""
import sys, os
sys.path.insert(0, '/opt/trn_rl_repo')

import numpy as np

# ---- problem constants (hardcoded per contract) ----
N_NODES = 100000
N_EDGES = 6400000
D_IN, D_F = 512, 16
NC = 8                   # cores
NPC_REAL = 12500         # real nodes per core
DPC = 320                # dst slots per chunk
NCHUNK = 40              # chunks per core
NPC = DPC * NCHUNK       # padded nodes per core (12800)
NPAD = NPC * NC          # padded global nodes (102400)
NGROUP = 8               # 16-partition groups per core
SRC_CHUNKS = 4           # src chunks (tables)
SRCW = NPAD // SRC_CHUNKS  # 25600 nodes per src chunk


def _pad_id(n):
    """original node id -> padded id"""
    return (n // NPC_REAL) * NPC + (n % NPC_REAL)


def _host_prep(x, edge_index, edge_weight):
    """Returns per-core input dicts + the shared chunk-size schedule."""
    src = np.asarray(edge_index[0], dtype=np.int64)
    dst = np.asarray(edge_index[1], dtype=np.int64)
    w = np.asarray(edge_weight, dtype=np.float32)

    spad = _pad_id(src)
    dcore = dst // NPC_REAL
    dloc = dst % NPC_REAL                    # 0..12499 local dst
    chunk = dloc // DPC                      # 0..39
    dslot = dloc % DPC                       # 0..319
    schunk = spad // SRCW                    # 0..3
    # subgroup split: alternate edges of the same src chunk between the
    # two groups that hold that table copy (keeps both streams dst-sorted)
    order0 = np.lexsort((dslot, chunk, schunk, dcore))
    sub = np.zeros(N_EDGES, dtype=np.int64)
    sub[order0] = np.arange(N_EDGES) % 2
    group = schunk * 2 + sub                 # 0..7

    # stream key per edge: (core, group, chunk, dslot)
    order = np.lexsort((dslot, chunk, group, dcore))
    so_src = spad[order]; so_w = w[order]
    so_core = dcore[order]; so_grp = group[order]
    so_chunk = chunk[order]; so_slot = dslot[order]

    # counts per (core, group, chunk)
    cgc = (so_core * NGROUP + so_grp) * NCHUNK + so_chunk
    counts = np.bincount(cgc, minlength=NC * NGROUP * NCHUNK).reshape(NC, NGROUP, NCHUNK)
    # shared schedule: C_k = 1 (pad slot 0) + max count, rounded to 64
    C = 1 + counts.max(axis=(0, 1))          # per chunk
    C = ((C + 63) // 64) * 64
    C_off = np.concatenate([[0], np.cumsum(C)])
    TOT = int(C_off[-1])

    # end-position (inclusive-prefix index) per (core,group,chunk,dslot):
    # bidx = cumulative count of edges with slot <= j  (pad at pos 0)
    cgcs = ((so_core * NGROUP + so_grp) * NCHUNK + so_chunk) * DPC + so_slot
    slot_counts = np.bincount(cgcs, minlength=NC * NGROUP * NCHUNK * DPC)
    slot_counts = slot_counts.reshape(NC, NGROUP, NCHUNK, DPC)
    bpos = np.cumsum(slot_counts, axis=3)    # int64 [NC,NG,NK,DPC]

    # edge positions within padded streams
    # within-chunk index of each sorted edge:
    cgc_sorted_off = np.concatenate([[0], np.cumsum(np.bincount(cgc, minlength=NC * NGROUP * NCHUNK))])
    within = np.arange(N_EDGES) - cgc_sorted_off[cgc]

    idx_all = np.zeros((NC, NGROUP, TOT), dtype=np.int16)
    w_all = np.zeros((NC, NGROUP, TOT), dtype=np.float32)
    pos = C_off[so_chunk] + 1 + within       # +1 for pad slot
    lin = (so_core * NGROUP + so_grp) * TOT + pos
    idx_flat = idx_all.reshape(-1); w_flat = w_all.reshape(-1)
    idx_flat[lin] = (so_src - (so_grp // 2) * SRCW).astype(np.int16)
    w_flat[lin] = so_w

    # wrap idx into 16 partitions: part 16g+j holds list[j::16]
    idx_wr = np.zeros((NC, 128, TOT // 16), dtype=np.int16)
    w_rep = np.zeros((NC, 128, TOT), dtype=np.float32)
    for g in range(NGROUP):
        for j in range(16):
            idx_wr[:, 16 * g + j, :] = idx_all[:, g, j::16]
        w_rep[:, 16 * g:16 * g + 16, :] = w_all[:, g, None, :]

    # boundary idx wrapped: per chunk 320 positions -> [128, 20] per chunk
    bidx_wr = np.zeros((NC, 128, NCHUNK * (DPC // 16)), dtype=np.int16)
    for g in range(NGROUP):
        for j in range(16):
            bidx_wr[:, 16 * g + j, :] = bpos[:, g, :, j::16].reshape(NC, -1).astype(np.int16)

    # xT slices [512, NPC] padded
    xT = np.zeros((NC, D_IN, NPC), dtype=np.float32)
    xf = np.asarray(x, dtype=np.float32)
    for c in range(NC):
        xT[c, :, :NPC_REAL] = xf[c * NPC_REAL:(c + 1) * NPC_REAL, :].T

    return idx_wr, w_rep, bidx_wr, xT, C, TOT


def _build_program(C, TOT, W1, b1, W2, b2):
    import concourse.bass as bass
    import concourse.bacc as bacc
    import concourse.mybir as mybir
    from concourse.tile import TileContext

    f32 = mybir.dt.float32
    i16 = mybir.dt.int16
    AO = mybir.AluOpType
    C_off = np.concatenate([[0], np.cumsum(C)]).astype(int)

    nc = bacc.Bacc("TRN2", target_bir_lowering=False, debug=False, num_devices=NC)

    # inputs
    xT_d = nc.dram_tensor("xT", [D_IN, NPC], f32, kind="ExternalInput")
    idx_d = nc.dram_tensor("idx", [128, TOT // 16], i16, kind="ExternalInput")
    w_d = nc.dram_tensor("w", [128, TOT], f32, kind="ExternalInput")
    bidx_d = nc.dram_tensor("bidx", [128, NCHUNK * (DPC // 16)], i16, kind="ExternalInput")
    W1_d = nc.dram_tensor("W1", [D_IN, D_F], f32, kind="ExternalInput")
    W2_d = nc.dram_tensor("W2", [D_F, D_F], f32, kind="ExternalInput")
    b1_d = nc.dram_tensor("b1", [D_F, 1], f32, kind="ExternalInput")
    b2_d = nc.dram_tensor("b2", [D_F, 1], f32, kind="ExternalInput")
    sel_d = nc.dram_tensor("sel", [128, D_F], f32, kind="ExternalInput")
    ones16_d = nc.dram_tensor("ones16", [D_F, 1], f32, kind="ExternalInput")
    one1_d = nc.dram_tensor("one1", [1, D_F], f32, kind="ExternalInput")
    id16_d = nc.dram_tensor("id16", [D_F, D_F], f32, kind="ExternalInput")
    out_d = nc.dram_tensor("out", [NPC, D_F], f32, kind="ExternalOutput")
    dbg = os.environ.get("GNN_DEBUG") == "1"
    if dbg:
        d_t1s = nc.dram_tensor("d_t1s", [D_F, NPC], f32, kind="ExternalOutput")
        d_tbl = nc.dram_tensor("d_tbl", [128, SRCW], f32, kind="ExternalOutput")
        d_gt0 = nc.dram_tensor("d_gt0", [128, 2688], f32, kind="ExternalOutput")
        d_pt0 = nc.dram_tensor("d_pt0", [128, 2688], f32, kind="ExternalOutput")
        d_bv0 = nc.dram_tensor("d_bv0", [128, DPC], f32, kind="ExternalOutput")
        d_agg0 = nc.dram_tensor("d_agg0", [D_F, DPC], f32, kind="ExternalOutput")
        d_t2s = nc.dram_tensor("d_t2s", [D_F, NPC], f32, kind="ExternalOutput")
        d_tbl2 = nc.dram_tensor("d_tbl2", [128, SRCW], f32, kind="ExternalOutput")
        d_agg2 = nc.dram_tensor("d_agg2", [D_F, DPC], f32, kind="ExternalOutput")
        d_zsb = nc.dram_tensor("d_zsb", [D_F, DPC], f32, kind="ExternalOutput")
        d_lsb = nc.dram_tensor("d_lsb", [1, DPC], f32, kind="ExternalOutput")

    # internal DRAM for slices + allgathered tables
    t1s = nc.dram_tensor("t1s", [D_F, NPC], f32)
    t2s = nc.dram_tensor("t2s", [D_F, NPC], f32)
    t1f = nc.dram_tensor("t1f", [NC * D_F, NPC], f32, addr_space="Shared")
    t2f = nc.dram_tensor("t2f", [NC * D_F, NPC], f32, addr_space="Shared")

    with TileContext(nc) as tc:
        with tc.tile_pool(name="const", bufs=1) as cpool:
            w1t = cpool.tile([128, 4, D_F], f32)
            for kp in range(4):
                nc.sync.dma_start(out=w1t[:, kp, :], in_=W1_d[kp * 128:(kp + 1) * 128, :])
            w2t = cpool.tile([D_F, D_F], f32)
            nc.sync.dma_start(out=w2t[:], in_=W2_d[:])
            b1t = cpool.tile([D_F, 1], f32)
            nc.sync.dma_start(out=b1t[:], in_=b1_d[:])
            b2t = cpool.tile([D_F, 1], f32)
            nc.sync.dma_start(out=b2t[:], in_=b2_d[:])
            selt = cpool.tile([128, D_F], f32)
            nc.sync.dma_start(out=selt[:], in_=sel_d[:])
            ones16t = cpool.tile([D_F, 1], f32)
            nc.sync.dma_start(out=ones16t[:], in_=ones16_d[:])
            one1t = cpool.tile([1, D_F], f32)
            nc.sync.dma_start(out=one1t[:], in_=one1_d[:])
            id16t = cpool.tile([D_F, D_F], f32)
            nc.sync.dma_start(out=id16t[:], in_=id16_d[:])

            # ---------------- phase A: t1 = W1^T @ xT  ----------------
            with (tc.tile_pool(name="pA", bufs=3) as pa,
                  tc.tile_pool(name="pAp", bufs=2, space="PSUM") as pap):
                for k in range(NCHUNK):
                    ps = pap.tile([D_F, DPC], f32, tag="t1ps")
                    for kp in range(4):
                        xt = pa.tile([128, DPC], f32, tag="xt")
                        nc.sync.dma_start(out=xt[:], in_=xT_d[kp * 128:(kp + 1) * 128,
                                                             k * DPC:(k + 1) * DPC])
                        nc.tensor.matmul(ps[:], lhsT=w1t[:, kp, :], rhs=xt[:],
                                         start=(kp == 0), stop=(kp == 3))
                    t1c = pa.tile([D_F, DPC], f32, tag="t1c")
                    nc.vector.tensor_copy(t1c[:], ps[:])
                    nc.sync.dma_start(out=t1s[:, k * DPC:(k + 1) * DPC], in_=t1c[:])

            nc.gpsimd.collective_compute(
                "AllGather", AO.bypass, replica_groups=[list(range(NC))],
                ins=[t1s[:]], outs=[t1f[:]],
            )

            def aggregate(tbl_full, layer):
                """aggregate pass over the edge streams; epilogue per layer."""
                with (tc.tile_pool(name=f"tblp{layer}", bufs=1) as tp,
                      tc.tile_pool(name=f"ed{layer}", bufs=2) as ep,
                      tc.tile_pool(name=f"m{layer}", bufs=1) as mp,
                      tc.tile_pool(name=f"agg{layer}", bufs=2, space="PSUM") as ap_,
                      tc.tile_pool(name=f"ps{layer}", bufs=1, space="PSUM") as pp):
                    tbl = tp.tile([128, SRCW], f32)
                    # group g table = src chunk g//2 = rows of 2 cores
                    for g in range(NGROUP):
                        sc = g // 2
                        for ci in range(2):
                            core_row = (sc * 2 + ci) * D_F
                            nc.sync.dma_start(
                                out=tbl[16 * g:16 * g + 16, ci * NPC:(ci + 1) * NPC],
                                in_=tbl_full[core_row:core_row + D_F, :])
                    for k in range(NCHUNK):
                        Ck = int(C[k]); o0 = int(C_off[k])
                        idxt = ep.tile([128, Ck // 16], i16, tag="idxt")
                        nc.sync.dma_start(out=idxt[:], in_=idx_d[:, o0 // 16:(o0 + Ck) // 16])
                        wt = ep.tile([128, Ck], f32, tag="wt")
                        nc.sync.dma_start(out=wt[:], in_=w_d[:, o0:o0 + Ck])
                        bit = ep.tile([128, DPC // 16], i16, tag="bit")
                        nc.sync.dma_start(out=bit[:], in_=bidx_d[:, k * (DPC // 16):(k + 1) * (DPC // 16)])

                        gt = ep.tile([128, Ck], f32, tag="gt")
                        nc.gpsimd.ap_gather(gt[:], tbl[:], idxt[:], channels=128,
                                            num_elems=SRCW, d=1, num_idxs=Ck)
                        mt = mp.tile([128, Ck], f32, tag="mt")
                        nc.vector.tensor_tensor(out=mt[:], in0=gt[:], in1=wt[:], op=AO.mult)
                        pt = ep.tile([128, Ck], f32, tag="pt")
                        nc.vector.tensor_tensor_scan(pt[:], mt[:], mt[:], 0.0, AO.add, AO.bypass)
                        bv = ep.tile([128, DPC], f32, tag="bv")
                        nc.gpsimd.ap_gather(bv[:], pt[:], bit[:], channels=128,
                                            num_elems=Ck, d=1, num_idxs=DPC)
                        dv = ep.tile([128, DPC], f32, tag="dv")
                        nc.vector.tensor_copy(dv[:, 0:1], bv[:, 0:1])
                        nc.vector.tensor_tensor(out=dv[:, 1:DPC], in0=bv[:, 1:DPC],
                                                in1=bv[:, 0:DPC - 1], op=AO.subtract)
                        agg = ap_.tile([D_F, DPC], f32, tag="agg")
                        nc.tensor.matmul(agg[:], lhsT=selt[:], rhs=dv[:], start=True, stop=True)
                        if dbg and layer == 1 and k == 0:
                            nc.sync.dma_start(out=d_tbl[:], in_=tbl[:])
                            nc.sync.dma_start(out=d_gt0[:, :Ck], in_=gt[:])
                            nc.sync.dma_start(out=d_pt0[:, :Ck], in_=pt[:])
                            nc.sync.dma_start(out=d_bv0[:], in_=bv[:])
                            aggc = ep.tile([D_F, DPC], f32, tag="aggc")
                            nc.vector.tensor_copy(aggc[:], agg[:])
                            nc.sync.dma_start(out=d_agg0[:], in_=aggc[:])

                        if dbg and layer == 2 and k == 0:
                            nc.sync.dma_start(out=d_tbl2[:], in_=tbl[:])
                            agg2c = ep.tile([D_F, DPC], f32, tag="agg2c")
                            nc.vector.tensor_copy(agg2c[:], agg[:])
                            nc.sync.dma_start(out=d_agg2[:], in_=agg2c[:])
                        if layer == 1:
                            # h = relu(agg + b1) -> t2s chunk
                            hc = ep.tile([D_F, DPC], f32, tag="hc")
                            nc.vector.tensor_scalar(out=hc[:], in0=agg[:], scalar1=b1t[:],
                                                    scalar2=0.0, op0=AO.add, op1=AO.max)
                            nc.sync.dma_start(out=t2s[:, k * DPC:(k + 1) * DPC], in_=hc[:])
                        else:
                            # z = W2^T agg + b2 ; out = z - log(sum(exp z))
                            asb = ep.tile([D_F, DPC], f32, tag="asb")
                            nc.vector.tensor_copy(asb[:], agg[:])
                            zps = pp.tile([D_F, DPC], f32, tag="zps")
                            nc.tensor.matmul(zps[:], lhsT=w2t[:], rhs=asb[:], start=True, stop=True)
                            zsb = ep.tile([D_F, DPC], f32, tag="zsb")
                            nc.vector.tensor_scalar(out=zsb[:], in0=zps[:], scalar1=b2t[:],
                                                    scalar2=None, op0=AO.add)
                            if dbg and k == 0:
                                nc.sync.dma_start(out=d_zsb[:], in_=zsb[:])
                            # transpose to node-major, then stable log_softmax
                            for j0 in range(0, DPC, 128):
                                bw = min(128, DPC - j0)
                                tps = pp.tile([128, D_F], f32, tag="tps")
                                nc.tensor.transpose(tps[:bw, :], zsb[:, j0:j0 + bw], id16t[:])
                                zt = ep.tile([128, D_F], f32, tag="zt")
                                nc.vector.tensor_copy(zt[:bw, :], tps[:bw, :])
                                mx = ep.tile([128, 1], f32, tag="mx")
                                nc.vector.reduce_max(mx[:bw, :], zt[:bw, :],
                                                     axis=mybir.AxisListType.X)
                                zs = ep.tile([128, D_F], f32, tag="zs")
                                nc.vector.tensor_scalar(out=zs[:bw, :], in0=zt[:bw, :],
                                                        scalar1=mx[:bw, :], scalar2=None,
                                                        op0=AO.subtract)
                                ez = ep.tile([128, D_F], f32, tag="ez")
                                nc.scalar.activation(ez[:bw, :], zs[:bw, :],
                                                     mybir.ActivationFunctionType.Exp)
                                sm = ep.tile([128, 1], f32, tag="sm")
                                nc.vector.reduce_sum(sm[:bw, :], ez[:bw, :],
                                                     axis=mybir.AxisListType.X)
                                ls = ep.tile([128, 1], f32, tag="ls")
                                nc.scalar.activation(ls[:bw, :], sm[:bw, :],
                                                     mybir.ActivationFunctionType.Ln)
                                ot = ep.tile([128, D_F], f32, tag="ot")
                                nc.vector.tensor_scalar(out=ot[:bw, :], in0=zs[:bw, :],
                                                        scalar1=ls[:bw, :], scalar2=None,
                                                        op0=AO.subtract)
                                nc.sync.dma_start(
                                    out=out_d[k * DPC + j0:k * DPC + j0 + bw, :],
                                    in_=ot[:bw, :])

            if dbg:
                with tc.tile_pool(name="dbgp", bufs=1) as dp:
                    tt = dp.tile([D_F, NPC], f32)
                    nc.sync.dma_start(out=tt[:], in_=t1s[:])
                    nc.sync.dma_start(out=d_t1s[:], in_=tt[:])
            aggregate(t1f, 1)
            if dbg:
                with tc.tile_pool(name="dbgp2", bufs=1) as dp2:
                    tt2 = dp2.tile([D_F, NPC], f32)
                    nc.sync.dma_start(out=tt2[:], in_=t2s[:])
                    nc.sync.dma_start(out=d_t2s[:], in_=tt2[:])
            nc.gpsimd.collective_compute(
                "AllGather", AO.bypass, replica_groups=[list(range(NC))],
                ins=[t2s[:]], outs=[t2f[:]],
            )
            aggregate(t2f, 2)

    nc.compile()
    return nc


def kernel(x, edge_index, edge_weight, W1, b1, W2, b2):
    from concourse.bass_utils import run_bass_kernel_spmd

    idx_wr, w_rep, bidx_wr, xT, C, TOT = _host_prep(x, edge_index, edge_weight)
    W1n = np.asarray(W1, np.float32); W2n = np.asarray(W2, np.float32)
    b1n = np.asarray(b1, np.float32).reshape(D_F, 1)
    b2n = np.asarray(b2, np.float32).reshape(D_F, 1)
    sel = np.zeros((128, D_F), np.float32)
    for g in range(NGROUP):
        for f in range(D_F):
            sel[16 * g + f, f] = 1.0
    ones16 = np.ones((D_F, 1), np.float32)
    one1 = np.ones((1, D_F), np.float32)
    id16 = np.eye(D_F, dtype=np.float32)

    nc = _build_program(C, TOT, W1n, b1n, W2n, b2n)

    in_maps = []
    for c in range(NC):
        in_maps.append({
            "xT": xT[c], "idx": idx_wr[c], "w": w_rep[c], "bidx": bidx_wr[c],
            "W1": W1n, "W2": W2n, "b1": b1n, "b2": b2n,
            "sel": sel, "ones16": ones16, "one1": one1, "id16": id16,
        })
    res = run_bass_kernel_spmd(nc, in_maps, list(range(NC)))
    out = np.zeros((N_NODES, D_F), np.float32)
    for c in range(NC):
        out[c * NPC_REAL:(c + 1) * NPC_REAL] = res.results[c]["out"][:NPC_REAL]
    return out


# revision 8
# speedup vs baseline: 1786.4888x; 1786.4888x over previous
"""2-layer GCN (gather/scatter message passing) on 8 trn2 NeuronCores.

Strategy (per sharding hint): nodes (and their incoming edges) are
partitioned across the 8 cores by dst-node range; each core computes
x@W1 for its node slice, slices are exchanged via AllGather (the
16-dim halo exchange), and each core aggregates messages for its dst
range twice (layer 1 and layer 2). Weight matrices are replicated.

On-chip per pass: the gather h[src] runs on GPSIMD (ap_gather) from
feature-major tables [16 feats x src-chunk] replicated per 16-partition
group; messages are weighted (DVE mul), prefix-summed along the
dst-sorted edge stream (DVE tensor_tensor_scan), and per-dst segment
sums are extracted by gathering the prefix at segment-end boundaries
and differencing (scan-diff).  Partials from the 8 groups are summed
with one PE matmul against a 0/1 selection matrix.
"""
import sys, os
sys.path.insert(0, '/opt/trn_rl_repo')

import numpy as np

# ---- problem constants (hardcoded per contract) ----
N_NODES = 100000
N_EDGES = 6400000
D_IN, D_F = 512, 16
NC = 8                   # cores
NPC_REAL = 12500         # real nodes per core
DPC = 320                # dst slots per chunk
NCHUNK = 40              # chunks per core
NPC = DPC * NCHUNK       # padded nodes per core (12800)
NPAD = NPC * NC          # padded global nodes (102400)
NGROUP = 8               # 16-partition groups per core
SRC_CHUNKS = 4           # src chunks (tables)
SRCW = NPAD // SRC_CHUNKS  # 25600 nodes per src chunk


def _pad_id(n):
    """original node id -> padded id"""
    return (n // NPC_REAL) * NPC + (n % NPC_REAL)


def _host_prep(x, edge_index, edge_weight):
    """Returns per-core input dicts + the shared chunk-size schedule."""
    src = np.asarray(edge_index[0], dtype=np.int64)
    dst = np.asarray(edge_index[1], dtype=np.int64)
    w = np.asarray(edge_weight, dtype=np.float32)

    spad = _pad_id(src)
    dcore = dst // NPC_REAL
    dloc = dst % NPC_REAL                    # 0..12499 local dst
    chunk = dloc // DPC                      # 0..39
    dslot = dloc % DPC                       # 0..319
    schunk = spad // SRCW                    # 0..3
    # subgroup split: alternate edges of the same src chunk between the
    # two groups that hold that table copy (keeps both streams dst-sorted)
    order0 = np.lexsort((dslot, chunk, schunk, dcore))
    sub = np.zeros(N_EDGES, dtype=np.int64)
    sub[order0] = np.arange(N_EDGES) % 2
    group = schunk * 2 + sub                 # 0..7

    # stream key per edge: (core, group, chunk, dslot)
    order = np.lexsort((dslot, chunk, group, dcore))
    so_src = spad[order]; so_w = w[order]
    so_core = dcore[order]; so_grp = group[order]
    so_chunk = chunk[order]; so_slot = dslot[order]

    # counts per (core, group, chunk)
    cgc = (so_core * NGROUP + so_grp) * NCHUNK + so_chunk
    counts = np.bincount(cgc, minlength=NC * NGROUP * NCHUNK).reshape(NC, NGROUP, NCHUNK)
    # shared schedule: C_k = 1 (pad slot 0) + max count, rounded to 64
    C = 1 + counts.max(axis=(0, 1))          # per chunk
    C = ((C + 63) // 64) * 64
    C_off = np.concatenate([[0], np.cumsum(C)])
    TOT = int(C_off[-1])

    # end-position (inclusive-prefix index) per (core,group,chunk,dslot):
    # bidx = cumulative count of edges with slot <= j  (pad at pos 0)
    cgcs = ((so_core * NGROUP + so_grp) * NCHUNK + so_chunk) * DPC + so_slot
    slot_counts = np.bincount(cgcs, minlength=NC * NGROUP * NCHUNK * DPC)
    slot_counts = slot_counts.reshape(NC, NGROUP, NCHUNK, DPC)
    bpos = np.cumsum(slot_counts, axis=3)    # int64 [NC,NG,NK,DPC]

    # edge positions within padded streams
    # within-chunk index of each sorted edge:
    cgc_sorted_off = np.concatenate([[0], np.cumsum(np.bincount(cgc, minlength=NC * NGROUP * NCHUNK))])
    within = np.arange(N_EDGES) - cgc_sorted_off[cgc]

    idx_all = np.zeros((NC, NGROUP, TOT), dtype=np.int16)
    w_all = np.zeros((NC, NGROUP, TOT), dtype=np.float32)
    pos = C_off[so_chunk] + 1 + within       # +1 for pad slot
    lin = (so_core * NGROUP + so_grp) * TOT + pos
    idx_flat = idx_all.reshape(-1); w_flat = w_all.reshape(-1)
    idx_flat[lin] = (so_src - (so_grp // 2) * SRCW).astype(np.int16)
    w_flat[lin] = so_w

    # wrap idx into 16 partitions: part 16g+j holds list[j::16]
    idx_wr = np.ascontiguousarray(
        idx_all.reshape(NC, NGROUP, TOT // 16, 16).transpose(0, 1, 3, 2)
    ).reshape(NC, 128, TOT // 16)
    w_rep = np.repeat(w_all, 16, axis=1)

    # boundary idx wrapped: per chunk DPC positions -> [128, DPC//16] per chunk
    bidx_wr = np.ascontiguousarray(
        bpos.astype(np.int16).reshape(NC, NGROUP, NCHUNK, DPC // 16, 16)
        .transpose(0, 1, 4, 2, 3)
    ).reshape(NC, 128, NCHUNK * (DPC // 16))

    # xT slices [512, NPC] padded
    xT = np.zeros((NC, D_IN, NPC), dtype=np.float32)
    xf = np.asarray(x, dtype=np.float32)
    for c in range(NC):
        xT[c, :, :NPC_REAL] = xf[c * NPC_REAL:(c + 1) * NPC_REAL, :].T

    return idx_wr, w_rep, bidx_wr, xT, C, TOT


def _build_program(C, TOT, W1, b1, W2, b2):
    import concourse.bass as bass
    import concourse.bacc as bacc
    import concourse.mybir as mybir
    from concourse.tile import TileContext

    f32 = mybir.dt.float32
    i16 = mybir.dt.int16
    AO = mybir.AluOpType
    C_off = np.concatenate([[0], np.cumsum(C)]).astype(int)

    nc = bacc.Bacc("TRN2", target_bir_lowering=False, debug=False, num_devices=NC)

    # inputs
    xT_d = nc.dram_tensor("xT", [D_IN, NPC], f32, kind="ExternalInput")
    idx_d = nc.dram_tensor("idx", [128, TOT // 16], i16, kind="ExternalInput")
    w_d = nc.dram_tensor("w", [128, TOT], f32, kind="ExternalInput")
    bidx_d = nc.dram_tensor("bidx", [128, NCHUNK * (DPC // 16)], i16, kind="ExternalInput")
    W1_d = nc.dram_tensor("W1", [D_IN, D_F], f32, kind="ExternalInput")
    W2_d = nc.dram_tensor("W2", [D_F, D_F], f32, kind="ExternalInput")
    b1_d = nc.dram_tensor("b1", [D_F, 1], f32, kind="ExternalInput")
    b2_d = nc.dram_tensor("b2", [D_F, 1], f32, kind="ExternalInput")
    sel_d = nc.dram_tensor("sel", [128, D_F], f32, kind="ExternalInput")
    ones16_d = nc.dram_tensor("ones16", [D_F, 1], f32, kind="ExternalInput")
    one1_d = nc.dram_tensor("one1", [1, D_F], f32, kind="ExternalInput")
    id16_d = nc.dram_tensor("id16", [D_F, D_F], f32, kind="ExternalInput")
    out_d = nc.dram_tensor("out", [NPC, D_F], f32, kind="ExternalOutput")
    dbg = os.environ.get("GNN_DEBUG") == "1"
    if dbg:
        d_t1s = nc.dram_tensor("d_t1s", [D_F, NPC], f32, kind="ExternalOutput")
        d_tbl = nc.dram_tensor("d_tbl", [128, SRCW], f32, kind="ExternalOutput")
        d_gt0 = nc.dram_tensor("d_gt0", [128, 2688], f32, kind="ExternalOutput")
        d_pt0 = nc.dram_tensor("d_pt0", [128, 2688], f32, kind="ExternalOutput")
        d_bv0 = nc.dram_tensor("d_bv0", [128, DPC], f32, kind="ExternalOutput")
        d_agg0 = nc.dram_tensor("d_agg0", [D_F, DPC], f32, kind="ExternalOutput")
        d_t2s = nc.dram_tensor("d_t2s", [D_F, NPC], f32, kind="ExternalOutput")
        d_tbl2 = nc.dram_tensor("d_tbl2", [128, SRCW], f32, kind="ExternalOutput")
        d_agg2 = nc.dram_tensor("d_agg2", [D_F, DPC], f32, kind="ExternalOutput")
        d_zsb = nc.dram_tensor("d_zsb", [D_F, DPC], f32, kind="ExternalOutput")
        d_lsb = nc.dram_tensor("d_lsb", [1, DPC], f32, kind="ExternalOutput")

    # internal DRAM for slices + allgathered tables
    t1s = nc.dram_tensor("t1s", [D_F, NPC], f32)
    t2s = nc.dram_tensor("t2s", [D_F, NPC], f32)
    t1f = nc.dram_tensor("t1f", [NC * D_F, NPC], f32, addr_space="Shared")
    t2f = nc.dram_tensor("t2f", [NC * D_F, NPC], f32, addr_space="Shared")

    with TileContext(nc) as tc:
        with tc.tile_pool(name="const", bufs=1) as cpool:
            w1t = cpool.tile([128, 4, D_F], f32)
            for kp in range(4):
                nc.sync.dma_start(out=w1t[:, kp, :], in_=W1_d[kp * 128:(kp + 1) * 128, :])
            w2t = cpool.tile([D_F, D_F], f32)
            nc.sync.dma_start(out=w2t[:], in_=W2_d[:])
            b1t = cpool.tile([D_F, 1], f32)
            nc.sync.dma_start(out=b1t[:], in_=b1_d[:])
            b2t = cpool.tile([D_F, 1], f32)
            nc.sync.dma_start(out=b2t[:], in_=b2_d[:])
            selt = cpool.tile([128, D_F], f32)
            nc.sync.dma_start(out=selt[:], in_=sel_d[:])
            ones16t = cpool.tile([D_F, 1], f32)
            nc.sync.dma_start(out=ones16t[:], in_=ones16_d[:])
            one1t = cpool.tile([1, D_F], f32)
            nc.sync.dma_start(out=one1t[:], in_=one1_d[:])
            id16t = cpool.tile([D_F, D_F], f32)
            nc.sync.dma_start(out=id16t[:], in_=id16_d[:])

            # ---------------- phase A: t1 = W1^T @ xT  ----------------
            with (tc.tile_pool(name="pA", bufs=3) as pa,
                  tc.tile_pool(name="pAp", bufs=2, space="PSUM") as pap):
                for k in range(NCHUNK):
                    ps = pap.tile([D_F, DPC], f32, tag="t1ps")
                    for kp in range(4):
                        xt = pa.tile([128, DPC], f32, tag="xt")
                        nc.sync.dma_start(out=xt[:], in_=xT_d[kp * 128:(kp + 1) * 128,
                                                             k * DPC:(k + 1) * DPC])
                        nc.tensor.matmul(ps[:], lhsT=w1t[:, kp, :], rhs=xt[:],
                                         start=(kp == 0), stop=(kp == 3))
                    t1c = pa.tile([D_F, DPC], f32, tag="t1c")
                    nc.vector.tensor_copy(t1c[:], ps[:])
                    nc.sync.dma_start(out=t1s[:, k * DPC:(k + 1) * DPC], in_=t1c[:])

            nc.gpsimd.collective_compute(
                "AllGather", AO.bypass, replica_groups=[list(range(NC))],
                ins=[t1s[:]], outs=[t1f[:]],
            )

            def aggregate(tbl_full, layer):
                """aggregate pass over the edge streams; epilogue per layer."""
                with (tc.tile_pool(name=f"tblp{layer}", bufs=1) as tp,
                      tc.tile_pool(name=f"ed{layer}", bufs=2) as ep,
                      tc.tile_pool(name=f"m{layer}", bufs=1) as mp,
                      tc.tile_pool(name=f"agg{layer}", bufs=2, space="PSUM") as ap_,
                      tc.tile_pool(name=f"ps{layer}", bufs=1, space="PSUM") as pp):
                    tbl = tp.tile([128, SRCW], f32)
                    # group g table = src chunk g//2 = rows of 2 cores
                    for g in range(NGROUP):
                        sc = g // 2
                        for ci in range(2):
                            core_row = (sc * 2 + ci) * D_F
                            nc.sync.dma_start(
                                out=tbl[16 * g:16 * g + 16, ci * NPC:(ci + 1) * NPC],
                                in_=tbl_full[core_row:core_row + D_F, :])
                    for k in range(NCHUNK):
                        Ck = int(C[k]); o0 = int(C_off[k])
                        idxt = ep.tile([128, Ck // 16], i16, tag="idxt")
                        nc.sync.dma_start(out=idxt[:], in_=idx_d[:, o0 // 16:(o0 + Ck) // 16])
                        wt = ep.tile([128, Ck], f32, tag="wt")
                        nc.sync.dma_start(out=wt[:], in_=w_d[:, o0:o0 + Ck])
                        bit = ep.tile([128, DPC // 16], i16, tag="bit")
                        nc.sync.dma_start(out=bit[:], in_=bidx_d[:, k * (DPC // 16):(k + 1) * (DPC // 16)])

                        gt = ep.tile([128, Ck], f32, tag="gt")
                        nc.gpsimd.ap_gather(gt[:], tbl[:], idxt[:], channels=128,
                                            num_elems=SRCW, d=1, num_idxs=Ck)
                        mt = mp.tile([128, Ck], f32, tag="mt")
                        nc.vector.tensor_tensor(out=mt[:], in0=gt[:], in1=wt[:], op=AO.mult)
                        pt = ep.tile([128, Ck], f32, tag="pt")
                        nc.vector.tensor_tensor_scan(pt[:], mt[:], mt[:], 0.0, AO.add, AO.bypass)
                        bv = ep.tile([128, DPC], f32, tag="bv")
                        nc.gpsimd.ap_gather(bv[:], pt[:], bit[:], channels=128,
                                            num_elems=Ck, d=1, num_idxs=DPC)
                        dv = ep.tile([128, DPC], f32, tag="dv")
                        nc.vector.tensor_copy(dv[:, 0:1], bv[:, 0:1])
                        nc.vector.tensor_tensor(out=dv[:, 1:DPC], in0=bv[:, 1:DPC],
                                                in1=bv[:, 0:DPC - 1], op=AO.subtract)
                        agg = ap_.tile([D_F, DPC], f32, tag="agg")
                        nc.tensor.matmul(agg[:], lhsT=selt[:], rhs=dv[:], start=True, stop=True)
                        if dbg and layer == 1 and k == 0:
                            nc.sync.dma_start(out=d_tbl[:], in_=tbl[:])
                            nc.sync.dma_start(out=d_gt0[:, :Ck], in_=gt[:])
                            nc.sync.dma_start(out=d_pt0[:, :Ck], in_=pt[:])
                            nc.sync.dma_start(out=d_bv0[:], in_=bv[:])
                            aggc = ep.tile([D_F, DPC], f32, tag="aggc")
                            nc.vector.tensor_copy(aggc[:], agg[:])
                            nc.sync.dma_start(out=d_agg0[:], in_=aggc[:])

                        if dbg and layer == 2 and k == 0:
                            nc.sync.dma_start(out=d_tbl2[:], in_=tbl[:])
                            agg2c = ep.tile([D_F, DPC], f32, tag="agg2c")
                            nc.vector.tensor_copy(agg2c[:], agg[:])
                            nc.sync.dma_start(out=d_agg2[:], in_=agg2c[:])
                        if layer == 1:
                            # h = relu(agg + b1) -> t2s chunk
                            hc = ep.tile([D_F, DPC], f32, tag="hc")
                            nc.vector.tensor_scalar(out=hc[:], in0=agg[:], scalar1=b1t[:],
                                                    scalar2=0.0, op0=AO.add, op1=AO.max)
                            nc.sync.dma_start(out=t2s[:, k * DPC:(k + 1) * DPC], in_=hc[:])
                        else:
                            # z = W2^T agg + b2 ; out = z - log(sum(exp z))
                            asb = ep.tile([D_F, DPC], f32, tag="asb")
                            nc.vector.tensor_copy(asb[:], agg[:])
                            zps = pp.tile([D_F, DPC], f32, tag="zps")
                            nc.tensor.matmul(zps[:], lhsT=w2t[:], rhs=asb[:], start=True, stop=True)
                            zsb = ep.tile([D_F, DPC], f32, tag="zsb")
                            nc.vector.tensor_scalar(out=zsb[:], in0=zps[:], scalar1=b2t[:],
                                                    scalar2=None, op0=AO.add)
                            if dbg and k == 0:
                                nc.sync.dma_start(out=d_zsb[:], in_=zsb[:])
                            # transpose to node-major, then stable log_softmax
                            for j0 in range(0, DPC, 128):
                                bw = min(128, DPC - j0)
                                tps = pp.tile([128, D_F], f32, tag="tps")
                                nc.tensor.transpose(tps[:bw, :], zsb[:, j0:j0 + bw], id16t[:])
                                zt = ep.tile([128, D_F], f32, tag="zt")
                                nc.vector.tensor_copy(zt[:bw, :], tps[:bw, :])
                                mx = ep.tile([128, 1], f32, tag="mx")
                                nc.vector.reduce_max(mx[:bw, :], zt[:bw, :],
                                                     axis=mybir.AxisListType.X)
                                zs = ep.tile([128, D_F], f32, tag="zs")
                                nc.vector.tensor_scalar(out=zs[:bw, :], in0=zt[:bw, :],
                                                        scalar1=mx[:bw, :], scalar2=None,
                                                        op0=AO.subtract)
                                ez = ep.tile([128, D_F], f32, tag="ez")
                                nc.scalar.activation(ez[:bw, :], zs[:bw, :],
                                                     mybir.ActivationFunctionType.Exp)
                                sm = ep.tile([128, 1], f32, tag="sm")
                                nc.vector.reduce_sum(sm[:bw, :], ez[:bw, :],
                                                     axis=mybir.AxisListType.X)
                                ls = ep.tile([128, 1], f32, tag="ls")
                                nc.scalar.activation(ls[:bw, :], sm[:bw, :],
                                                     mybir.ActivationFunctionType.Ln)
                                ot = ep.tile([128, D_F], f32, tag="ot")
                                nc.vector.tensor_scalar(out=ot[:bw, :], in0=zs[:bw, :],
                                                        scalar1=ls[:bw, :], scalar2=None,
                                                        op0=AO.subtract)
                                nc.sync.dma_start(
                                    out=out_d[k * DPC + j0:k * DPC + j0 + bw, :],
                                    in_=ot[:bw, :])

            if dbg:
                with tc.tile_pool(name="dbgp", bufs=1) as dp:
                    tt = dp.tile([D_F, NPC], f32)
                    nc.sync.dma_start(out=tt[:], in_=t1s[:])
                    nc.sync.dma_start(out=d_t1s[:], in_=tt[:])
            aggregate(t1f, 1)
            if dbg:
                with tc.tile_pool(name="dbgp2", bufs=1) as dp2:
                    tt2 = dp2.tile([D_F, NPC], f32)
                    nc.sync.dma_start(out=tt2[:], in_=t2s[:])
                    nc.sync.dma_start(out=d_t2s[:], in_=tt2[:])
            nc.gpsimd.collective_compute(
                "AllGather", AO.bypass, replica_groups=[list(range(NC))],
                ins=[t2s[:]], outs=[t2f[:]],
            )
            aggregate(t2f, 2)

    nc.compile()
    return nc


def kernel(x, edge_index, edge_weight, W1, b1, W2, b2):
    from concourse.bass_utils import run_bass_kernel_spmd

    idx_wr, w_rep, bidx_wr, xT, C, TOT = _host_prep(x, edge_index, edge_weight)
    W1n = np.asarray(W1, np.float32); W2n = np.asarray(W2, np.float32)
    b1n = np.asarray(b1, np.float32).reshape(D_F, 1)
    b2n = np.asarray(b2, np.float32).reshape(D_F, 1)
    sel = np.zeros((128, D_F), np.float32)
    for g in range(NGROUP):
        for f in range(D_F):
            sel[16 * g + f, f] = 1.0
    ones16 = np.ones((D_F, 1), np.float32)
    one1 = np.ones((1, D_F), np.float32)
    id16 = np.eye(D_F, dtype=np.float32)

    nc = _build_program(C, TOT, W1n, b1n, W2n, b2n)

    in_maps = []
    for c in range(NC):
        in_maps.append({
            "xT": xT[c], "idx": idx_wr[c], "w": w_rep[c], "bidx": bidx_wr[c],
            "W1": W1n, "W2": W2n, "b1": b1n, "b2": b2n,
            "sel": sel, "ones16": ones16, "one1": one1, "id16": id16,
        })
    res = run_bass_kernel_spmd(nc, in_maps, list(range(NC)))
    out = np.zeros((N_NODES, D_F), np.float32)
    for c in range(NC):
        out[c * NPC_REAL:(c + 1) * NPC_REAL] = res.results[c]["out"][:NPC_REAL]
    return out
